# revision 32
# baseline (speedup 1.0000x reference)
"""Trainium2 kernel for nn_AttentionModule_34308198761142.

Strategy: the N^2 softmax attention over invariant channels dominates the
FLOPs (~8.6 GF of 4x4096x4096x32 matmuls). It runs on the 8 NeuronCores via
a Bass/Tile kernel, row-sharded: core c owns query rows [c*512,(c+1)*512) of
every batch. Scores are computed transposed (S^T[j,i] tiles, K=32
contraction) so the exp'd probabilities land in SBUF already in the [j,i]
layout the P@V contraction needs -- no transposes. The softmax denominator
comes for free from a ones-column appended to V. Cheap O(B*N*C) glue
(norms, 32x32 projections, FFT long-conv, gating MLP) runs on host numpy.
"""

import os
import numpy as np

B, N = 4, 4096
NCORES = 8
ROWS = N // NCORES          # 512 query rows per batch per core
EPS = 1e-6
SCALE = 1.0 / np.sqrt(np.float32(32.0))

LAST_RESULTS = None         # BassKernelResults of the most recent run
LAST_RUN_S = None           # wall seconds of the most recent SPMD dispatch
_NC_CACHE = {}


def _build_attention_bass():
    """S^T-layout flash attention, one i-block of 512 rows per batch."""
    import concourse.bass as bass
    import concourse.tile as tile
    from concourse import mybir
    from contextlib import ExitStack

    nc = bass.Bass()
    dt = mybir.dt.float32
    # kT/vA are identical on every core (replicated shard_map specs);
    # only qT is per-core. The BIR wait-splitter handles the extra DMA waits.
    kT = nc.dram_tensor("kT", [B, 32, N], dt, kind="ExternalInput")
    qT = nc.dram_tensor("qT", [B, 32, ROWS], dt, kind="ExternalInput")
    # pre-tiled on host: vA[b, p, t, d] = [V | ones][t*128+p, d]
    vA = nc.dram_tensor("vA", [B, 128, 32, 33], dt, kind="ExternalInput")
    # transposed output: rows 0..31 = (P@V)^T unnormalized, row 32 = sum(P)
    out = nc.dram_tensor("attn", [B, 33, ROWS], dt, kind="ExternalOutput")

    with ExitStack() as ctx:
        tc = ctx.enter_context(tile.TileContext(nc))
        const = ctx.enter_context(tc.tile_pool(name="const", bufs=2))
        sb = ctx.enter_context(tc.tile_pool(name="sb", bufs=4))
        ps = ctx.enter_context(tc.tile_pool(name="ps", bufs=3, space="PSUM"))
        po = ctx.enter_context(tc.tile_pool(name="po", bufs=1, space="PSUM"))
        outp = ctx.enter_context(tc.tile_pool(name="outp", bufs=16))

        for b in range(B):
            vA_sb = const.tile([128, 32, 33], dt, tag="vA")
            nc.sync.dma_start(out=vA_sb, in_=vA[b])
            kT_sb = const.tile([32, N], dt, tag="kT")
            nc.sync.dma_start(out=kT_sb, in_=kT[b])
            qT_sb = const.tile([32, ROWS], dt, tag="qT")
            nc.sync.dma_start(out=qT_sb, in_=qT[b])

            # wait-absorbers: soak up the DMA-queue waits in one accumulation
            # group so the real matmuls below see the data as already-synced
            # (PE program order); extra waits are hoisted by the BIR splitter.
            dm1 = po.tile([1, 1], dt, name="dm1", tag="dmy")
            nc.tensor.matmul(
                dm1, lhsT=kT_sb[:, 0:1], rhs=qT_sb[:, 0:1],
                start=True, stop=False,
            )
            nc.tensor.matmul(
                dm1, lhsT=vA_sb[0:32, 0, 0:1], rhs=vA_sb[0:32, 0, 0:1],
                start=False, stop=True,
            )

            # single accumulator: out^T[d,i] += V_tile^T-stationary @ P^T
            pv_ps = po.tile([33, ROWS], dt, name="pv", tag="pv")
            for jt in range(32):
                s_ps = ps.tile([128, ROWS], dt)
                nc.tensor.matmul(
                    s_ps,
                    lhsT=kT_sb[:, jt * 128:(jt + 1) * 128],
                    rhs=qT_sb,
                    start=True,
                    stop=True,
                )
                # exp(scores/sqrt(32)); scores ~N(0,1) so no max-shift needed
                pt = sb.tile([128, ROWS], dt)
                nc.scalar.activation(
                    pt, s_ps, mybir.ActivationFunctionType.Exp, scale=float(SCALE)
                )
                nc.tensor.matmul(
                    pv_ps,
                    lhsT=vA_sb[:, jt, :],
                    rhs=pt,
                    start=(jt == 0),
                    stop=(jt == 31),
                )
            o_sb = outp.tile([33, ROWS], dt, tag="ocp")
            # ACT (not DVE) copy: keeps the psum slot's release on the
            # same semaphore the PV start-matmul already waits on
            nc.scalar.activation(
                o_sb, pv_ps, mybir.ActivationFunctionType.Copy
            )
            nc.sync.dma_start(out=out[b], in_=o_sb)
    return nc


def _install_wait_splitter():
    """This walrus build allows a single sem-wait per engine instruction;
    Tile emits up to 2 (plus an 11-wait Drain). Rewrite the BIR so every
    extra wait becomes a standalone EventSemaphore on the same engine."""
    import json as _json
    import concourse.bass2jax as b2j

    if getattr(b2j, "_wait_split_installed", False):
        return
    orig = b2j.compile_bir_kernel

    def patched(bir_json, tmpdir, neff_name="file.neff"):
        bir = _json.loads(bir_json)
        for fn in bir.get("functions", []):
            for blk in fn.get("blocks", []):
                new = []
                for inst in blk.get("instructions", []):
                    si = inst.get("sync_info") or {}
                    w = si.get("on_wait") or []
                    if len(w) > 1:
                        for j, wx in enumerate(w[:-1]):
                            new.append({
                                "debug": inst.get("debug", 0),
                                "engine": inst["engine"],
                                "ins": [],
                                "outs": [],
                                "name": f"{inst['name']}_w{j}",
                                "opcode": "EventSemaphore",
                                "sync_info": {"on_update": [], "on_wait": [wx]},
                            })
                        si["on_wait"] = [w[-1]]
                    new.append(inst)
                blk["instructions"] = new
        return orig(_json.dumps(bir).encode(), tmpdir, neff_name=neff_name)

    b2j.compile_bir_kernel = patched
    b2j._wait_split_installed = True


def _get_sharded():
    """Build (once) and cache the jitted shard_map callable, skipping
    run_bass_via_pjrt's per-call retrace/lower (~0.9 s of the warm path)."""
    if "sharded" in _NC_CACHE:
        return _NC_CACHE["sharded"]
    import jax
    from jax.experimental.shard_map import shard_map
    from jax.sharding import Mesh, PartitionSpec
    import concourse.bass2jax as b2j
    import concourse.mybir as mybir

    b2j.install_neuronx_cc_hook()
    nc = _NC_CACHE.setdefault("nc", _build_attention_bass())
    pname = nc.partition_id_tensor.name if nc.partition_id_tensor else None

    in_names, out_names, out_avals, zero_outs = [], [], [], []
    for alloc in nc.m.functions[0].allocations:
        if not isinstance(alloc, mybir.MemoryLocationSet):
            continue
        name = alloc.memorylocations[0].name
        if alloc.kind == "ExternalInput":
            if name != pname:
                in_names.append(name)
        elif alloc.kind == "ExternalOutput":
            shape = tuple(alloc.tensor_shape)
            dtype = mybir.dt.np(alloc.dtype)
            out_names.append(name)
            out_avals.append(jax.core.ShapedArray(shape, dtype))
            zero_outs.append(np.zeros(shape, dtype))
    n_params = len(in_names)
    all_names = in_names + out_names + ([pname] if pname else [])
    donate = tuple(range(n_params, n_params + len(out_names)))

    def _body(*args):
        operands = list(args)
        if pname:
            operands.append(b2j.partition_id_tensor())
        return tuple(b2j._bass_exec_p.bind(
            *operands,
            out_avals=tuple(out_avals),
            in_names=tuple(all_names),
            out_names=tuple(out_names),
            lowering_input_output_aliases=(),
            sim_require_finite=True,
            sim_require_nnan=True,
            nc=nc,
        ))

    mesh = Mesh(np.asarray(jax.devices()[:NCORES]), ("core",))
    core, repl = PartitionSpec("core"), PartitionSpec()
    # kT/vA are broadcast; qT and the outputs are per-core
    sharded_names = {"qT"}
    in_specs = tuple(
        core if nm in sharded_names else repl for nm in in_names
    ) + (core,) * len(out_names)
    sharded = jax.jit(
        shard_map(
            _body, mesh=mesh,
            in_specs=in_specs,
            out_specs=(core,) * len(out_names),
            check_rep=False,
        ),
        donate_argnums=donate,
        keep_unused=True,
    )
    _NC_CACHE["sharded"] = (
        sharded, in_names, sharded_names, out_names, out_avals, zero_outs
    )
    return _NC_CACHE["sharded"]


def _attention_hw(q_inv, k_inv, v_inv):
    """softmax(QK^T/sqrt(32)) @ V on the 8 NeuronCores."""
    global LAST_RESULTS
    from concourse import bass_utils

    _install_wait_splitter()

    if "nc" not in _NC_CACHE:
        _NC_CACHE["nc"] = _build_attention_bass()
    nc = _NC_CACHE["nc"]

    kT = np.ascontiguousarray(k_inv.transpose(0, 2, 1), np.float32)   # [B,32,N]
    qT_all = np.ascontiguousarray(q_inv.transpose(0, 2, 1), np.float32)
    vA = np.concatenate(
        [v_inv, np.ones((B, N, 1), np.float32)], axis=-1
    ).astype(np.float32)                                              # [B,N,33]
    # pre-tile for the kernel's [p, t, d] SBUF layout
    vA = np.ascontiguousarray(
        vA.reshape(B, 32, 128, 33).transpose(0, 2, 1, 3)
    )                                                                 # [B,128,32,33]
    in_maps = []
    for c in range(NCORES):
        in_maps.append({
            "kT": kT,
            "qT": np.ascontiguousarray(qT_all[:, :, c * ROWS:(c + 1) * ROWS]),
            "vA": vA,
        })
    import time as _time
    global LAST_RUN_S
    _t0 = _time.time()
    try:
        (sharded, in_names, sharded_names, out_names, out_avals,
         zero_outs) = _get_sharded()
        concat_in = [
            np.concatenate([m[nm] for m in in_maps], axis=0)
            if nm in sharded_names else in_maps[0][nm]
            for nm in in_names
        ]
        concat_zeros = [
            np.zeros((NCORES * z.shape[0], *z.shape[1:]), z.dtype)
            for z in zero_outs
        ]
        out_arrs = sharded(*concat_in, *concat_zeros)
        per_core = np.asarray(out_arrs[0]).reshape(
            NCORES, *out_avals[0].shape
        )  # [NCORES, B, 33, ROWS]
        results = [{"attn": per_core[c]} for c in range(NCORES)]
    except Exception:
        r = bass_utils.run_bass_kernel_spmd(
            nc, in_maps, core_ids=list(range(NCORES))
        )
        LAST_RESULTS = r
        results = r.results
    LAST_RUN_S = _time.time() - _t0
    u_inv = np.empty((B, N, 32), np.float32)
    for c in range(NCORES):
        a = results[c]["attn"]                       # [B, 33, ROWS]
        u_inv[:, c * ROWS:(c + 1) * ROWS, :] = (
            a[:, :32, :] / a[:, 32:33, :]
        ).transpose(0, 2, 1)
    return u_inv


def _batch_norm(x, f, mult):
    mu = f.mean(-1, keepdims=True)
    var = f.var(-1, keepdims=True)
    f = (f - mu) / np.sqrt(var + EPS)
    b, n, _ = x.shape
    xv = x.reshape(b, n, mult, 3)
    norms = np.sqrt((xv * xv).sum(-1) + EPS)
    scale = norms.mean(1, keepdims=True)
    xv = xv / (scale[..., None] + EPS)
    return xv.reshape(b, n, 3 * mult), f


def _lin_proj(x, f, Wv, Wi, bi, m_in):
    b, n, _ = x.shape
    xv = x.reshape(b, n, m_in, 3)
    xo = np.einsum("bnmi,om->bnoi", xv, Wv)
    return xo.reshape(b, n, -1), f @ Wi + bi


def _norm_act(x, f, mult):
    b, n, _ = x.shape
    xv = x.reshape(b, n, mult, 3)
    nrm = np.sqrt((xv * xv).sum(-1, keepdims=True) + EPS)
    xv = xv * (1.0 / (1.0 + np.exp(-nrm)))
    return xv.reshape(b, n, 3 * mult), np.maximum(f, 0.0)


def _softmax(a):
    e = np.exp(a - a.max(-1, keepdims=True))
    return e / e.sum(-1, keepdims=True)


def kernel(x, f, Wv_q, Wi_q, bi_q, Wv_k, Wi_k, bi_k, Wv_v, Wi_v, bi_v,
           Wg1, bg1, Wg2, bg2, Wv_p1, Wi_p1, bi_p1, Wv_fp, Wi_fp, bi_fp):
    args = [x, f, Wv_q, Wi_q, bi_q, Wv_k, Wi_k, bi_k, Wv_v, Wi_v, bi_v,
            Wg1, bg1, Wg2, bg2, Wv_p1, Wi_p1, bi_p1, Wv_fp, Wi_fp, bi_fp]
    (x, f, Wv_q, Wi_q, bi_q, Wv_k, Wi_k, bi_k, Wv_v, Wi_v, bi_v,
     Wg1, bg1, Wg2, bg2, Wv_p1, Wi_p1, bi_p1, Wv_fp, Wi_fp, bi_fp) = [
        np.asarray(a, np.float32) for a in args]

    b, n, _ = x.shape
    x_res, f_res = x, f
    x, f = _batch_norm(x, f, 1)

    q_eqv, q_inv = _lin_proj(x, f, Wv_q, Wi_q, bi_q, 1)
    k_eqv, k_inv = _lin_proj(x, f, Wv_k, Wi_k, bi_k, 1)
    v_eqv, v_inv = _lin_proj(x, f, Wv_v, Wi_v, bi_v, 1)

    # scalar path: N^2 attention on the NeuronCores
    u_inv = _attention_hw(q_inv, k_inv, v_inv)

    # vector path: circular FFT long conv
    Fq = np.fft.rfft(q_eqv, axis=1)
    Fk = np.fft.rfft(k_eqv, axis=1)
    u_eqv = np.fft.irfft(Fq * Fk, n=n, axis=1).astype(np.float32)

    uv = u_eqv.reshape(b, n, 3, 3)
    vnorms = np.sqrt((uv * uv).sum(-1) + EPS)
    feats = np.concatenate([vnorms, u_inv], axis=-1)
    h = np.maximum(feats @ Wg1 + bg1, 0.0)
    g = h @ Wg2 + bg2
    g_vec, m_inv = g[..., :3], g[..., 3:]
    m_eqv = np.repeat(g_vec, 3, axis=-1)

    vv = v_eqv.reshape(b, n, 3, 3)
    cr = np.cross(uv, vv).reshape(b, n, 9)
    u_eqv = _softmax(m_eqv) * cr

    u_eqv, u_inv = _lin_proj(u_eqv, u_inv, Wv_p1, Wi_p1, bi_p1, 3)
    x = x_res + u_eqv
    f = f_res + u_inv

    x, f = _batch_norm(x, f, 1)
    x, f = _norm_act(x, f, 1)
    x, f = _lin_proj(x, f, Wv_fp, Wi_fp, bi_fp, 1)
    x, f = _batch_norm(x, f, 1)
    x, f = _norm_act(x, f, 1)
    return np.asarray(x, np.float32), np.asarray(f, np.float32)


# revision 33
# speedup vs baseline: 1.0634x; 1.0634x over previous
"""Trainium2 kernel for nn_AttentionModule_34308198761142.

Strategy: the N^2 softmax attention over invariant channels dominates the
FLOPs (~8.6 GF of 4x4096x4096x32 matmuls). It runs on the 8 NeuronCores via
a Bass/Tile kernel, row-sharded: core c owns query rows [c*512,(c+1)*512) of
every batch. Scores are computed transposed (S^T[j,i] tiles, K=32
contraction) so the exp'd probabilities land in SBUF already in the [j,i]
layout the P@V contraction needs -- no transposes. The softmax denominator
comes for free from a ones-column appended to V. Cheap O(B*N*C) glue
(norms, 32x32 projections, FFT long-conv, gating MLP) runs on host numpy.
"""

import os
import numpy as np

B, N = 4, 4096
NCORES = 8
ROWS = N // NCORES          # 512 query rows per batch per core
EPS = 1e-6
SCALE = 1.0 / np.sqrt(np.float32(32.0))

LAST_RESULTS = None         # BassKernelResults of the most recent run
LAST_RUN_S = None           # wall seconds of the most recent SPMD dispatch
_NC_CACHE = {}


def _build_attention_bass():
    """S^T-layout flash attention, one i-block of 512 rows per batch."""
    import concourse.bass as bass
    import concourse.tile as tile
    from concourse import mybir
    from contextlib import ExitStack

    nc = bass.Bass()
    dt = mybir.dt.float32
    # kT/vA are identical on every core (replicated shard_map specs);
    # only qT is per-core. The BIR wait-splitter handles the extra DMA waits.
    kT = nc.dram_tensor("kT", [B, 32, N], dt, kind="ExternalInput")
    qT = nc.dram_tensor("qT", [B, 32, ROWS], dt, kind="ExternalInput")
    # pre-tiled on host: vA[b, p, t, d] = [V | ones][t*128+p, d]
    vA = nc.dram_tensor("vA", [B, 128, 32, 33], dt, kind="ExternalInput")
    # transposed output: rows 0..31 = (P@V)^T unnormalized, row 32 = sum(P)
    out = nc.dram_tensor("attn", [B, 33, ROWS], dt, kind="ExternalOutput")

    with ExitStack() as ctx:
        tc = ctx.enter_context(tile.TileContext(nc))
        const = ctx.enter_context(tc.tile_pool(name="const", bufs=2))
        sb = ctx.enter_context(tc.tile_pool(name="sb", bufs=4))
        ps = ctx.enter_context(tc.tile_pool(name="ps", bufs=3, space="PSUM"))
        po = ctx.enter_context(tc.tile_pool(name="po", bufs=1, space="PSUM"))
        outp = ctx.enter_context(tc.tile_pool(name="outp", bufs=16))

        for b in range(B):
            vA_sb = const.tile([128, 32, 33], dt, tag="vA")
            nc.sync.dma_start(out=vA_sb, in_=vA[b])
            kT_sb = const.tile([32, N], dt, tag="kT")
            nc.sync.dma_start(out=kT_sb, in_=kT[b])
            qT_sb = const.tile([32, ROWS], dt, tag="qT")
            nc.sync.dma_start(out=qT_sb, in_=qT[b])

            # wait-absorbers: soak up the DMA-queue waits in one accumulation
            # group so the real matmuls below see the data as already-synced
            # (PE program order); extra waits are hoisted by the BIR splitter.
            dm1 = po.tile([1, 1], dt, name="dm1", tag="dmy")
            nc.tensor.matmul(
                dm1, lhsT=kT_sb[:, 0:1], rhs=qT_sb[:, 0:1],
                start=True, stop=False,
            )
            nc.tensor.matmul(
                dm1, lhsT=vA_sb[0:32, 0, 0:1], rhs=vA_sb[0:32, 0, 0:1],
                start=False, stop=True,
            )

            # single accumulator: out^T[d,i] += V_tile^T-stationary @ P^T
            pv_ps = po.tile([33, ROWS], dt, name="pv", tag="pv")
            for jt2 in range(16):
                # two j-tiles of scores share one 2-bank psum tile so a
                # single wide exp amortizes the per-op ACT overhead
                s_ps = ps.tile([128, 2 * ROWS], dt)
                for h in range(2):
                    jt = jt2 * 2 + h
                    nc.tensor.matmul(
                        s_ps[:, h * ROWS:(h + 1) * ROWS],
                        lhsT=kT_sb[:, jt * 128:(jt + 1) * 128],
                        rhs=qT_sb,
                        start=True,
                        stop=True,
                    )
                # exp(scores/sqrt(32)); scores ~N(0,1) so no max-shift needed
                pt = sb.tile([128, 2 * ROWS], dt)
                nc.scalar.activation(
                    pt, s_ps, mybir.ActivationFunctionType.Exp, scale=float(SCALE)
                )
                for h in range(2):
                    jt = jt2 * 2 + h
                    nc.tensor.matmul(
                        pv_ps,
                        lhsT=vA_sb[:, jt, :],
                        rhs=pt[:, h * ROWS:(h + 1) * ROWS],
                        start=(jt == 0),
                        stop=(jt == 31),
                    )
            o_sb = outp.tile([33, ROWS], dt, tag="ocp")
            # ACT (not DVE) copy: keeps the psum slot's release on the
            # same semaphore the PV start-matmul already waits on
            nc.scalar.activation(
                o_sb, pv_ps, mybir.ActivationFunctionType.Copy
            )
            nc.sync.dma_start(out=out[b], in_=o_sb)
    return nc


def _install_wait_splitter():
    """This walrus build allows a single sem-wait per engine instruction;
    Tile emits up to 2 (plus an 11-wait Drain). Rewrite the BIR so every
    extra wait becomes a standalone EventSemaphore on the same engine."""
    import json as _json
    import concourse.bass2jax as b2j

    if getattr(b2j, "_wait_split_installed", False):
        return
    orig = b2j.compile_bir_kernel

    def patched(bir_json, tmpdir, neff_name="file.neff"):
        bir = _json.loads(bir_json)
        for fn in bir.get("functions", []):
            for blk in fn.get("blocks", []):
                new = []
                for inst in blk.get("instructions", []):
                    si = inst.get("sync_info") or {}
                    w = si.get("on_wait") or []
                    if len(w) > 1:
                        for j, wx in enumerate(w[:-1]):
                            new.append({
                                "debug": inst.get("debug", 0),
                                "engine": inst["engine"],
                                "ins": [],
                                "outs": [],
                                "name": f"{inst['name']}_w{j}",
                                "opcode": "EventSemaphore",
                                "sync_info": {"on_update": [], "on_wait": [wx]},
                            })
                        si["on_wait"] = [w[-1]]
                    new.append(inst)
                blk["instructions"] = new
        return orig(_json.dumps(bir).encode(), tmpdir, neff_name=neff_name)

    b2j.compile_bir_kernel = patched
    b2j._wait_split_installed = True


def _get_sharded():
    """Build (once) and cache the jitted shard_map callable, skipping
    run_bass_via_pjrt's per-call retrace/lower (~0.9 s of the warm path)."""
    if "sharded" in _NC_CACHE:
        return _NC_CACHE["sharded"]
    import jax
    from jax.experimental.shard_map import shard_map
    from jax.sharding import Mesh, PartitionSpec
    import concourse.bass2jax as b2j
    import concourse.mybir as mybir

    b2j.install_neuronx_cc_hook()
    nc = _NC_CACHE.setdefault("nc", _build_attention_bass())
    pname = nc.partition_id_tensor.name if nc.partition_id_tensor else None

    in_names, out_names, out_avals, zero_outs = [], [], [], []
    for alloc in nc.m.functions[0].allocations:
        if not isinstance(alloc, mybir.MemoryLocationSet):
            continue
        name = alloc.memorylocations[0].name
        if alloc.kind == "ExternalInput":
            if name != pname:
                in_names.append(name)
        elif alloc.kind == "ExternalOutput":
            shape = tuple(alloc.tensor_shape)
            dtype = mybir.dt.np(alloc.dtype)
            out_names.append(name)
            out_avals.append(jax.core.ShapedArray(shape, dtype))
            zero_outs.append(np.zeros(shape, dtype))
    n_params = len(in_names)
    all_names = in_names + out_names + ([pname] if pname else [])
    donate = tuple(range(n_params, n_params + len(out_names)))

    def _body(*args):
        operands = list(args)
        if pname:
            operands.append(b2j.partition_id_tensor())
        return tuple(b2j._bass_exec_p.bind(
            *operands,
            out_avals=tuple(out_avals),
            in_names=tuple(all_names),
            out_names=tuple(out_names),
            lowering_input_output_aliases=(),
            sim_require_finite=True,
            sim_require_nnan=True,
            nc=nc,
        ))

    mesh = Mesh(np.asarray(jax.devices()[:NCORES]), ("core",))
    core, repl = PartitionSpec("core"), PartitionSpec()
    # kT/vA are broadcast; qT and the outputs are per-core
    sharded_names = {"qT"}
    in_specs = tuple(
        core if nm in sharded_names else repl for nm in in_names
    ) + (core,) * len(out_names)
    sharded = jax.jit(
        shard_map(
            _body, mesh=mesh,
            in_specs=in_specs,
            out_specs=(core,) * len(out_names),
            check_rep=False,
        ),
        donate_argnums=donate,
        keep_unused=True,
    )
    _NC_CACHE["sharded"] = (
        sharded, in_names, sharded_names, out_names, out_avals, zero_outs
    )
    return _NC_CACHE["sharded"]


def _attention_hw(q_inv, k_inv, v_inv):
    """softmax(QK^T/sqrt(32)) @ V on the 8 NeuronCores."""
    global LAST_RESULTS
    from concourse import bass_utils

    _install_wait_splitter()

    if "nc" not in _NC_CACHE:
        _NC_CACHE["nc"] = _build_attention_bass()
    nc = _NC_CACHE["nc"]

    kT = np.ascontiguousarray(k_inv.transpose(0, 2, 1), np.float32)   # [B,32,N]
    qT_all = np.ascontiguousarray(q_inv.transpose(0, 2, 1), np.float32)
    vA = np.concatenate(
        [v_inv, np.ones((B, N, 1), np.float32)], axis=-1
    ).astype(np.float32)                                              # [B,N,33]
    # pre-tile for the kernel's [p, t, d] SBUF layout
    vA = np.ascontiguousarray(
        vA.reshape(B, 32, 128, 33).transpose(0, 2, 1, 3)
    )                                                                 # [B,128,32,33]
    in_maps = []
    for c in range(NCORES):
        in_maps.append({
            "kT": kT,
            "qT": np.ascontiguousarray(qT_all[:, :, c * ROWS:(c + 1) * ROWS]),
            "vA": vA,
        })
    import time as _time
    global LAST_RUN_S
    _t0 = _time.time()
    try:
        (sharded, in_names, sharded_names, out_names, out_avals,
         zero_outs) = _get_sharded()
        concat_in = [
            np.concatenate([m[nm] for m in in_maps], axis=0)
            if nm in sharded_names else in_maps[0][nm]
            for nm in in_names
        ]
        concat_zeros = [
            np.zeros((NCORES * z.shape[0], *z.shape[1:]), z.dtype)
            for z in zero_outs
        ]
        out_arrs = sharded(*concat_in, *concat_zeros)
        per_core = np.asarray(out_arrs[0]).reshape(
            NCORES, *out_avals[0].shape
        )  # [NCORES, B, 33, ROWS]
        results = [{"attn": per_core[c]} for c in range(NCORES)]
    except Exception:
        r = bass_utils.run_bass_kernel_spmd(
            nc, in_maps, core_ids=list(range(NCORES))
        )
        LAST_RESULTS = r
        results = r.results
    LAST_RUN_S = _time.time() - _t0
    u_inv = np.empty((B, N, 32), np.float32)
    for c in range(NCORES):
        a = results[c]["attn"]                       # [B, 33, ROWS]
        u_inv[:, c * ROWS:(c + 1) * ROWS, :] = (
            a[:, :32, :] / a[:, 32:33, :]
        ).transpose(0, 2, 1)
    return u_inv


def _batch_norm(x, f, mult):
    mu = f.mean(-1, keepdims=True)
    var = f.var(-1, keepdims=True)
    f = (f - mu) / np.sqrt(var + EPS)
    b, n, _ = x.shape
    xv = x.reshape(b, n, mult, 3)
    norms = np.sqrt((xv * xv).sum(-1) + EPS)
    scale = norms.mean(1, keepdims=True)
    xv = xv / (scale[..., None] + EPS)
    return xv.reshape(b, n, 3 * mult), f


def _lin_proj(x, f, Wv, Wi, bi, m_in):
    b, n, _ = x.shape
    xv = x.reshape(b, n, m_in, 3)
    xo = np.einsum("bnmi,om->bnoi", xv, Wv)
    return xo.reshape(b, n, -1), f @ Wi + bi


def _norm_act(x, f, mult):
    b, n, _ = x.shape
    xv = x.reshape(b, n, mult, 3)
    nrm = np.sqrt((xv * xv).sum(-1, keepdims=True) + EPS)
    xv = xv * (1.0 / (1.0 + np.exp(-nrm)))
    return xv.reshape(b, n, 3 * mult), np.maximum(f, 0.0)


def _softmax(a):
    e = np.exp(a - a.max(-1, keepdims=True))
    return e / e.sum(-1, keepdims=True)


def kernel(x, f, Wv_q, Wi_q, bi_q, Wv_k, Wi_k, bi_k, Wv_v, Wi_v, bi_v,
           Wg1, bg1, Wg2, bg2, Wv_p1, Wi_p1, bi_p1, Wv_fp, Wi_fp, bi_fp):
    args = [x, f, Wv_q, Wi_q, bi_q, Wv_k, Wi_k, bi_k, Wv_v, Wi_v, bi_v,
            Wg1, bg1, Wg2, bg2, Wv_p1, Wi_p1, bi_p1, Wv_fp, Wi_fp, bi_fp]
    (x, f, Wv_q, Wi_q, bi_q, Wv_k, Wi_k, bi_k, Wv_v, Wi_v, bi_v,
     Wg1, bg1, Wg2, bg2, Wv_p1, Wi_p1, bi_p1, Wv_fp, Wi_fp, bi_fp) = [
        np.asarray(a, np.float32) for a in args]

    b, n, _ = x.shape
    x_res, f_res = x, f
    x, f = _batch_norm(x, f, 1)

    q_eqv, q_inv = _lin_proj(x, f, Wv_q, Wi_q, bi_q, 1)
    k_eqv, k_inv = _lin_proj(x, f, Wv_k, Wi_k, bi_k, 1)
    v_eqv, v_inv = _lin_proj(x, f, Wv_v, Wi_v, bi_v, 1)

    # scalar path: N^2 attention on the NeuronCores
    u_inv = _attention_hw(q_inv, k_inv, v_inv)

    # vector path: circular FFT long conv
    Fq = np.fft.rfft(q_eqv, axis=1)
    Fk = np.fft.rfft(k_eqv, axis=1)
    u_eqv = np.fft.irfft(Fq * Fk, n=n, axis=1).astype(np.float32)

    uv = u_eqv.reshape(b, n, 3, 3)
    vnorms = np.sqrt((uv * uv).sum(-1) + EPS)
    feats = np.concatenate([vnorms, u_inv], axis=-1)
    h = np.maximum(feats @ Wg1 + bg1, 0.0)
    g = h @ Wg2 + bg2
    g_vec, m_inv = g[..., :3], g[..., 3:]
    m_eqv = np.repeat(g_vec, 3, axis=-1)

    vv = v_eqv.reshape(b, n, 3, 3)
    cr = np.cross(uv, vv).reshape(b, n, 9)
    u_eqv = _softmax(m_eqv) * cr

    u_eqv, u_inv = _lin_proj(u_eqv, u_inv, Wv_p1, Wi_p1, bi_p1, 3)
    x = x_res + u_eqv
    f = f_res + u_inv

    x, f = _batch_norm(x, f, 1)
    x, f = _norm_act(x, f, 1)
    x, f = _lin_proj(x, f, Wv_fp, Wi_fp, bi_fp, 1)
    x, f = _batch_norm(x, f, 1)
    x, f = _norm_act(x, f, 1)
    return np.asarray(x, np.float32), np.asarray(f, np.float32)
